# revision 2
# baseline (speedup 1.0000x reference)
"""AdaptiveGroupNorm (global mean/var over the whole tensor) on 8 TRN2 cores.

reference semantics (indexes == arange(N*C), so the gather/scatter is identity):
    mean = x.mean();  var = ((x - mean)**2).sum() / (x.size - 1)
    out  = (x - mean) / sqrt(var + eps) * weight + bias     (weight/bias per-channel)

Strategy: data-parallel over N (4 batches per core, 16 MiB/core kept fully in
SBUF).  Local Σx / Σx² are computed per-tile while load DMAs stream in, folded
across partitions with a ones-vector matmul, all-reduced (32 B) across the 8
cores, then each tile is normalized in place and stored.  HBM traffic per core
is exactly one read + one write of the shard.
"""

import numpy as np

import concourse.bass as bass
import concourse.bacc as bacc
import concourse.tile as tile
from concourse import mybir
from concourse.bass_utils import run_bass_kernel_spmd

N_CORES = 8
EPS = 1e-5
N, C, H, W = 32, 256, 64, 64
N_LOC = N // N_CORES            # 4 batches per core
ROWS = N_LOC * C                # 1024 (n,c) rows per core
F = H * W                       # 4096 elements per row
P = 128                         # partitions
NTILES = ROWS // P              # 8 SBUF tiles of (128, 4096)
CNT = N * C * H * W             # global element count
FP32 = mybir.dt.float32


def build_nc() -> bass.Bass:
    nc = bacc.Bacc("TRN2", target_bir_lowering=False, debug=False, num_devices=N_CORES)

    x_ext = nc.declare_dram_parameter("x", [N_LOC, C, H, W], FP32, isOutput=False)
    w_ext = nc.declare_dram_parameter("weight", [1, C, 1, 1], FP32, isOutput=False)
    b_ext = nc.declare_dram_parameter("bias", [1, C, 1, 1], FP32, isOutput=False)
    out_ext = nc.declare_dram_parameter("out", [N_LOC, C, H, W], FP32, isOutput=True)

    # (t, p, f) views: row r = t*128 + p maps to channel (r % 256), so even
    # tiles hold channels 0..127 and odd tiles channels 128..255.
    xv = x_ext.ap().rearrange("n c h w -> (n c) (h w)").rearrange("(t p) f -> t p f", p=P)
    ov = out_ext.ap().rearrange("n c h w -> (n c) (h w)").rearrange("(t p) f -> t p f", p=P)
    # weight/bias as (128, 2): col 0 = channels 0..127, col 1 = channels 128..255
    wv = w_ext.ap().rearrange("a c b d -> (a b d c)").rearrange("(t p) -> p t", p=P)
    bv = b_ext.ap().rearrange("a c b d -> (a b d c)").rearrange("(t p) -> p t", p=P)

    with tile.TileContext(nc, num_cores=N_CORES) as tc:
        with (
            tc.tile_pool(name="data", bufs=NTILES) as data,
            tc.tile_pool(name="scratch", bufs=2) as scratch,
            tc.tile_pool(name="small", bufs=1) as small,
            tc.tile_pool(name="psum", bufs=1, space="PSUM") as psum,
            tc.tile_pool(name="dram", bufs=1, space="DRAM") as dram,
        ):
            ones_t = small.tile([P, 1], FP32)
            nc.vector.memset(ones_t, 1.0)
            w_t = small.tile([P, 2], FP32)
            b_t = small.tile([P, 2], FP32)
            nc.sync.dma_start(out=w_t, in_=wv)
            nc.sync.dma_start(out=b_t, in_=bv)

            # cols 0..7: per-tile per-partition sum; cols 8..15: sum of squares
            parts = small.tile([P, 2 * NTILES], FP32)

            xts = []
            for t in range(NTILES):
                xt = data.tile([P, F], FP32)
                nc.sync.dma_start(out=xt, in_=xv[t])
                xts.append(xt)
                nc.vector.reduce_sum(
                    out=parts[:, t : t + 1], in_=xt, axis=mybir.AxisListType.X
                )
                sq = scratch.tile([P, F], FP32)
                nc.scalar.activation(
                    out=sq,
                    in_=xt,
                    func=mybir.ActivationFunctionType.Square,
                    accum_out=parts[:, NTILES + t : NTILES + t + 1],
                )

            # fold across partitions: ones(128,1)^T @ parts(128,16) -> (1,16)
            ps = psum.tile([1, 2 * NTILES], FP32)
            nc.tensor.matmul(ps, ones_t, parts, start=True, stop=True)

            cc_sb = small.tile([1, 8], FP32)
            nc.vector.memset(cc_sb, 0.0)
            nc.vector.reduce_sum(
                out=cc_sb[:, 0:2],
                in_=ps.rearrange("p (g k) -> p g k", g=2),
                axis=mybir.AxisListType.X,
            )

            cc_in = dram.tile([1, 8], FP32)
            cc_out = dram.tile([1, 8], FP32)
            nc.sync.dma_start(out=cc_in[:], in_=cc_sb)
            nc.gpsimd.collective_compute(
                "AllReduce",
                mybir.AluOpType.add,
                replica_groups=[list(range(N_CORES))],
                ins=[cc_in.opt()],
                outs=[cc_out.opt()],
            )

            # broadcast [S, SS] to all 128 partitions
            cc_ap = cc_out[:]
            stats_b = small.tile([P, 2], FP32)
            bc = bass.AP(tensor=cc_ap.tensor, offset=cc_ap.offset, ap=[[0, P], [1, 2]])
            nc.gpsimd.dma_start(out=stats_b, in_=bc)

            S = stats_b[:, 0:1]
            SS = stats_b[:, 1:2]
            mean = small.tile([P, 1], FP32)
            nc.vector.tensor_scalar_mul(out=mean, in0=S, scalar1=1.0 / CNT)
            t0 = small.tile([P, 1], FP32)
            nc.vector.tensor_mul(out=t0, in0=S, in1=mean)     # S^2/cnt
            e2 = small.tile([P, 1], FP32)
            nc.vector.tensor_sub(out=e2, in0=SS, in1=t0)      # Σ(x-mean)^2
            eps_t = small.tile([P, 1], FP32)
            nc.vector.memset(eps_t, EPS)
            std = small.tile([P, 1], FP32)
            nc.scalar.activation(                              # sqrt(E/(cnt-1)+eps)
                out=std,
                in_=e2,
                func=mybir.ActivationFunctionType.Sqrt,
                scale=1.0 / (CNT - 1),
                bias=eps_t,
            )
            rstd = small.tile([P, 1], FP32)
            nc.vector.reciprocal(out=rstd, in_=std)

            A_t = small.tile([P, 2], FP32)                     # w * rstd
            nc.vector.tensor_scalar_mul(out=A_t, in0=w_t, scalar1=rstd)
            mA = small.tile([P, 2], FP32)
            nc.vector.tensor_scalar_mul(out=mA, in0=A_t, scalar1=mean)
            B_t = small.tile([P, 2], FP32)                     # b - mean * A
            nc.vector.tensor_sub(out=B_t, in0=b_t, in1=mA)

            for t in range(NTILES):
                col = t % 2
                nc.vector.tensor_scalar(
                    out=xts[t],
                    in0=xts[t],
                    scalar1=A_t[:, col : col + 1],
                    scalar2=B_t[:, col : col + 1],
                    op0=mybir.AluOpType.mult,
                    op1=mybir.AluOpType.add,
                )
                nc.sync.dma_start(out=ov[t], in_=xts[t])

    nc.compile()
    return nc


_NC_CACHE: list = []


def _get_nc() -> bass.Bass:
    if not _NC_CACHE:
        _NC_CACHE.append(build_nc())
    return _NC_CACHE[0]


def kernel(x, weight, bias, indexes=None, **_unused):
    x = np.ascontiguousarray(np.asarray(x, dtype=np.float32))
    weight = np.ascontiguousarray(np.asarray(weight, dtype=np.float32).reshape(1, C, 1, 1))
    bias = np.ascontiguousarray(np.asarray(bias, dtype=np.float32).reshape(1, C, 1, 1))
    assert x.shape == (N, C, H, W)

    nc = _get_nc()
    in_maps = [
        {
            "x": np.ascontiguousarray(x[i * N_LOC : (i + 1) * N_LOC]),
            "weight": weight,
            "bias": bias,
        }
        for i in range(N_CORES)
    ]
    res = run_bass_kernel_spmd(nc, in_maps, core_ids=list(range(N_CORES)))
    out = np.concatenate([res.results[i]["out"] for i in range(N_CORES)], axis=0)
    return out


if __name__ == "__main__":
    nc = build_nc()
    print("build + compile OK:", nc)


# revision 5
# speedup vs baseline: 1.1885x; 1.1885x over previous
"""AdaptiveGroupNorm (global mean/var over the whole tensor) on 8 TRN2 cores.

reference semantics (indexes == arange(N*C), so the gather/scatter is identity):
    mean = x.mean();  var = ((x - mean)**2).sum() / (x.size - 1)
    out  = (x - mean) / sqrt(var + eps) * weight + bias     (weight/bias per-channel)

Strategy: data-parallel over N (4 batches per core, 16 MiB/core kept fully in
SBUF).  Local Σx / Σx² are computed per-tile while load DMAs stream in and
all-reduced in TWO stages (first half of the tiles early — that collective
absorbs the NEFF entry barrier and ncfw wakeup — second half right after the
last load), then each tile is normalized in place and stored.  HBM traffic per
core is exactly one read + one write of the shard.
"""

import numpy as np

import concourse.bass as bass
import concourse.bacc as bacc
import concourse.tile as tile
from concourse import mybir
from concourse.bass_utils import run_bass_kernel_spmd

N_CORES = 8
EPS = 1e-5
N, C, H, W = 32, 256, 64, 64
N_LOC = N // N_CORES            # 4 batches per core
ROWS = N_LOC * C                # 1024 (n,c) rows per core
F = H * W                       # 4096 elements per row
P = 128                         # partitions
NTILES = ROWS // P              # 8 logical row-tiles of (128, 4096)
CNT = N * C * H * W             # global element count
FP32 = mybir.dt.float32

# load/compute chunks: (row_tile, col_start, col_len). Last row-tile is split
# in half so the final chunk's stats land sooner after its load completes.
CHUNKS = [(t, 0, F) for t in range(NTILES - 1)] + [
    (NTILES - 1, 0, F // 2),
    (NTILES - 1, F // 2, F // 2),
]
HALF_A = [i for i, (t, _, _) in enumerate(CHUNKS) if t < NTILES // 2]   # tiles 0..3
HALF_B = [i for i, (t, _, _) in enumerate(CHUNKS) if t >= NTILES // 2]  # tiles 4..7


def build_nc() -> bass.Bass:
    nc = bacc.Bacc("TRN2", target_bir_lowering=False, debug=False, num_devices=N_CORES)

    x_ext = nc.declare_dram_parameter("x", [N_LOC, C, H, W], FP32, isOutput=False)
    w_ext = nc.declare_dram_parameter("weight", [1, C, 1, 1], FP32, isOutput=False)
    b_ext = nc.declare_dram_parameter("bias", [1, C, 1, 1], FP32, isOutput=False)
    out_ext = nc.declare_dram_parameter("out", [N_LOC, C, H, W], FP32, isOutput=True)

    # (t, p, f) views: row r = t*128 + p maps to channel (r % 256), so even
    # tiles hold channels 0..127 and odd tiles channels 128..255.
    xv = x_ext.ap().rearrange("n c h w -> (n c) (h w)").rearrange("(t p) f -> t p f", p=P)
    ov = out_ext.ap().rearrange("n c h w -> (n c) (h w)").rearrange("(t p) f -> t p f", p=P)
    # weight/bias as (128, 2): col 0 = channels 0..127, col 1 = channels 128..255
    wv = w_ext.ap().rearrange("a c b d -> (a b d c)").rearrange("(t p) -> p t", p=P)
    bv = b_ext.ap().rearrange("a c b d -> (a b d c)").rearrange("(t p) -> p t", p=P)

    replica = [list(range(N_CORES))]

    with tile.TileContext(nc, num_cores=N_CORES) as tc:
        with (
            tc.tile_pool(name="data", bufs=NTILES - 1) as data,
            tc.tile_pool(name="data_half", bufs=2) as data_half,
            tc.tile_pool(name="scratch", bufs=2) as scratch,
            tc.tile_pool(name="small", bufs=1) as small,
            tc.tile_pool(name="psum", bufs=2, space="PSUM") as psum,
            tc.tile_pool(name="dram", bufs=1, space="DRAM") as dram,
        ):
            ones_t = small.tile([P, 1], FP32)
            nc.vector.memset(ones_t, 1.0)
            eps_t = small.tile([P, 1], FP32)
            nc.vector.memset(eps_t, EPS)
            w_t = small.tile([P, 2], FP32)
            b_t = small.tile([P, 2], FP32)
            nc.sync.dma_start(out=w_t, in_=wv)
            nc.sync.dma_start(out=b_t, in_=bv)

            # per-half partial stats: cols 0..k-1 = per-chunk Σx,
            # cols k..2k-1 = per-chunk Σx²  (k = chunks in that half)
            ka, kb = len(HALF_A), len(HALF_B)
            parts_a = small.tile([P, 2 * ka], FP32)
            parts_b = small.tile([P, 2 * kb], FP32)

            chunk_tiles = []
            for ci, (t, c0, clen) in enumerate(CHUNKS):
                pool = data if clen == F else data_half
                xt = pool.tile([P, clen], FP32, tag="xt" if clen == F else "xth")
                nc.sync.dma_start(out=xt, in_=xv[t, :, c0 : c0 + clen])
                chunk_tiles.append(xt)
                if ci in HALF_A:
                    parts, k, j = parts_a, ka, HALF_A.index(ci)
                else:
                    parts, k, j = parts_b, kb, HALF_B.index(ci)
                nc.vector.reduce_sum(
                    out=parts[:, j : j + 1], in_=xt, axis=mybir.AxisListType.X
                )
                sq = scratch.tile([P, F], FP32, tag="sq")
                nc.scalar.activation(
                    out=sq[:, :clen],
                    in_=xt,
                    func=mybir.ActivationFunctionType.Square,
                    accum_out=parts[:, k + j : k + j + 1],
                )

            # fold each half across partitions and all-reduce it.  The first
            # collective fires while the second half is still loading.
            bcasts = []
            for name, parts, k in (("a", parts_a, ka), ("b", parts_b, kb)):
                ps = psum.tile([1, 2 * k], FP32, tag=f"ps_{name}")
                nc.tensor.matmul(ps, ones_t, parts, start=True, stop=True)
                cc_sb = small.tile([1, 8], FP32, tag=f"cc_sb_{name}")
                nc.vector.memset(cc_sb, 0.0)
                nc.vector.reduce_sum(
                    out=cc_sb[:, 0:2],
                    in_=ps.rearrange("p (g k) -> p g k", g=2),
                    axis=mybir.AxisListType.X,
                )
                cc_in = dram.tile([1, 8], FP32, tag=f"cc_in_{name}")
                cc_out = dram.tile([1, 8], FP32, tag=f"cc_out_{name}")
                nc.sync.dma_start(out=cc_in[:], in_=cc_sb)
                nc.gpsimd.collective_compute(
                    "AllReduce",
                    mybir.AluOpType.add,
                    replica_groups=replica,
                    ins=[cc_in.opt()],
                    outs=[cc_out.opt()],
                )
                # broadcast [S, SS] of this half to all 128 partitions
                cc_ap = cc_out[:]
                bc_src = bass.AP(
                    tensor=cc_ap.tensor, offset=cc_ap.offset, ap=[[0, P], [1, 2]]
                )
                bc = small.tile([P, 2], FP32, tag=f"bcast_{name}")
                nc.gpsimd.dma_start(out=bc, in_=bc_src)
                bcasts.append(bc)

            stats = small.tile([P, 2], FP32)            # [S, SS] global
            nc.vector.tensor_add(out=stats, in0=bcasts[0], in1=bcasts[1])
            S = stats[:, 0:1]
            SS = stats[:, 1:2]

            mean = small.tile([P, 1], FP32)             # ACT: S/cnt
            nc.scalar.activation(
                out=mean, in_=S, func=mybir.ActivationFunctionType.Copy,
                scale=1.0 / CNT,
            )
            t0 = small.tile([P, 1], FP32)               # DVE: S*S
            nc.vector.tensor_mul(out=t0, in0=S, in1=S)
            e2 = small.tile([P, 1], FP32)               # DVE: SS - S²/cnt
            nc.vector.tensor_scalar(
                out=e2, in0=t0, scalar1=-1.0 / CNT, scalar2=SS,
                op0=mybir.AluOpType.mult, op1=mybir.AluOpType.add,
            )
            std = small.tile([P, 1], FP32)              # ACT: sqrt(E/(cnt-1)+eps)
            nc.scalar.activation(
                out=std, in_=e2, func=mybir.ActivationFunctionType.Sqrt,
                scale=1.0 / (CNT - 1), bias=eps_t,
            )
            rstd = small.tile([P, 1], FP32)             # DVE
            nc.vector.reciprocal(out=rstd, in_=std)
            A_t = small.tile([P, 2], FP32)              # DVE: w * rstd
            nc.vector.tensor_scalar_mul(out=A_t, in0=w_t, scalar1=rstd)
            mA = small.tile([P, 2], FP32)               # ACT: A * mean
            nc.scalar.activation(
                out=mA, in_=A_t, func=mybir.ActivationFunctionType.Copy, scale=mean,
            )
            B_t = small.tile([P, 2], FP32)              # DVE: b - mean * A
            nc.vector.tensor_sub(out=B_t, in0=b_t, in1=mA)

            for ci, (t, c0, clen) in enumerate(CHUNKS):
                col = t % 2
                xt = chunk_tiles[ci]
                nc.vector.tensor_scalar(
                    out=xt,
                    in0=xt,
                    scalar1=A_t[:, col : col + 1],
                    scalar2=B_t[:, col : col + 1],
                    op0=mybir.AluOpType.mult,
                    op1=mybir.AluOpType.add,
                )
                nc.sync.dma_start(out=ov[t, :, c0 : c0 + clen], in_=xt)

    nc.compile()
    return nc


_NC_CACHE: list = []


def _get_nc() -> bass.Bass:
    if not _NC_CACHE:
        _NC_CACHE.append(build_nc())
    return _NC_CACHE[0]


def kernel(x, weight, bias, indexes=None, **_unused):
    x = np.ascontiguousarray(np.asarray(x, dtype=np.float32))
    weight = np.ascontiguousarray(np.asarray(weight, dtype=np.float32).reshape(1, C, 1, 1))
    bias = np.ascontiguousarray(np.asarray(bias, dtype=np.float32).reshape(1, C, 1, 1))
    assert x.shape == (N, C, H, W)

    nc = _get_nc()
    in_maps = [
        {
            "x": np.ascontiguousarray(x[i * N_LOC : (i + 1) * N_LOC]),
            "weight": weight,
            "bias": bias,
        }
        for i in range(N_CORES)
    ]
    res = run_bass_kernel_spmd(nc, in_maps, core_ids=list(range(N_CORES)))
    out = np.concatenate([res.results[i]["out"] for i in range(N_CORES)], axis=0)
    return out


if __name__ == "__main__":
    nc = build_nc()
    print("build + compile OK:", nc)


# revision 7
# speedup vs baseline: 1.2477x; 1.0498x over previous
"""AdaptiveGroupNorm (global mean/var over the whole tensor) on 8 TRN2 cores.

reference semantics (indexes == arange(N*C), so the gather/scatter is identity):
    mean = x.mean();  var = ((x - mean)**2).sum() / (x.size - 1)
    out  = (x - mean) / sqrt(var + eps) * weight + bias     (weight/bias per-channel)

Strategy: data-parallel over N (4 batches per core, 16 MiB/core kept fully in
SBUF).  Local Σx / Σx² are computed per-tile while load DMAs stream in and
all-reduced in TWO stages (first half of the tiles early — that collective
absorbs the NEFF entry barrier and ncfw wakeup — second half right after the
last load), then each tile is normalized in place and stored.  HBM traffic per
core is exactly one read + one write of the shard.
"""

import numpy as np

import concourse.bass as bass
import concourse.bacc as bacc
import concourse.tile as tile
from concourse import mybir
from concourse.bass_utils import run_bass_kernel_spmd

N_CORES = 8
EPS = 1e-5
N, C, H, W = 32, 256, 64, 64
N_LOC = N // N_CORES            # 4 batches per core
ROWS = N_LOC * C                # 1024 (n,c) rows per core
F = H * W                       # 4096 elements per row
P = 128                         # partitions
NTILES = ROWS // P              # 8 logical row-tiles of (128, 4096)
CNT = N * C * H * W             # global element count
FP32 = mybir.dt.float32

# load/compute chunks: (row_tile, col_start, col_len). Last row-tile is split
# in half so the final chunk's stats land sooner after its load completes.
CHUNKS = [(t, 0, F) for t in range(NTILES - 1)] + [
    (NTILES - 1, 0, F // 2),
    (NTILES - 1, F // 2, F // 2),
]
HALF_A = [i for i, (t, _, _) in enumerate(CHUNKS) if t < NTILES // 2]   # tiles 0..3
HALF_B = [i for i, (t, _, _) in enumerate(CHUNKS) if t >= NTILES // 2]  # tiles 4..7


def build_nc() -> bass.Bass:
    nc = bacc.Bacc("TRN2", target_bir_lowering=False, debug=False, num_devices=N_CORES)

    x_ext = nc.declare_dram_parameter("x", [N_LOC, C, H, W], FP32, isOutput=False)
    w_ext = nc.declare_dram_parameter("weight", [1, C, 1, 1], FP32, isOutput=False)
    b_ext = nc.declare_dram_parameter("bias", [1, C, 1, 1], FP32, isOutput=False)
    out_ext = nc.declare_dram_parameter("out", [N_LOC, C, H, W], FP32, isOutput=True)

    # (t, p, f) views: row r = t*128 + p maps to channel (r % 256), so even
    # tiles hold channels 0..127 and odd tiles channels 128..255.
    xv = x_ext.ap().rearrange("n c h w -> (n c) (h w)").rearrange("(t p) f -> t p f", p=P)
    ov = out_ext.ap().rearrange("n c h w -> (n c) (h w)").rearrange("(t p) f -> t p f", p=P)
    # weight/bias as (128, 2): col 0 = channels 0..127, col 1 = channels 128..255
    wv = w_ext.ap().rearrange("a c b d -> (a b d c)").rearrange("(t p) -> p t", p=P)
    bv = b_ext.ap().rearrange("a c b d -> (a b d c)").rearrange("(t p) -> p t", p=P)

    replica = [list(range(N_CORES))]

    with tile.TileContext(nc, num_cores=N_CORES) as tc:
        with (
            tc.tile_pool(name="data", bufs=NTILES - 1) as data,
            tc.tile_pool(name="data_half", bufs=2) as data_half,
            tc.tile_pool(name="scratch", bufs=2) as scratch,
            tc.tile_pool(name="small", bufs=1) as small,
            tc.tile_pool(name="psum", bufs=2, space="PSUM") as psum,
            tc.tile_pool(name="dram", bufs=1, space="DRAM") as dram,
        ):
            ones_t = small.tile([P, 1], FP32)
            nc.vector.memset(ones_t, 1.0)
            eps_t = small.tile([P, 1], FP32)
            nc.vector.memset(eps_t, EPS)
            w_t = small.tile([P, 2], FP32)
            b_t = small.tile([P, 2], FP32)
            nc.sync.dma_start(out=w_t, in_=wv)
            nc.sync.dma_start(out=b_t, in_=bv)

            # per-half partial stats: cols 0..k-1 = per-chunk Σx,
            # cols k..2k-1 = per-chunk Σx²  (k = chunks in that half)
            ka, kb = len(HALF_A), len(HALF_B)
            parts_a = small.tile([P, 2 * ka], FP32)
            parts_b = small.tile([P, 2 * kb], FP32)

            chunk_tiles = []
            for ci, (t, c0, clen) in enumerate(CHUNKS):
                pool = data if clen == F else data_half
                xt = pool.tile([P, clen], FP32, tag="xt" if clen == F else "xth")
                nc.sync.dma_start(out=xt, in_=xv[t, :, c0 : c0 + clen])
                chunk_tiles.append(xt)
                if ci in HALF_A:
                    parts, k, j = parts_a, ka, HALF_A.index(ci)
                else:
                    parts, k, j = parts_b, kb, HALF_B.index(ci)
                nc.vector.reduce_sum(
                    out=parts[:, j : j + 1], in_=xt, axis=mybir.AxisListType.X
                )
                sq = scratch.tile([P, F], FP32, tag="sq")
                nc.scalar.activation(
                    out=sq[:, :clen],
                    in_=xt,
                    func=mybir.ActivationFunctionType.Square,
                    accum_out=parts[:, k + j : k + j + 1],
                )

            # fold each half across partitions and all-reduce it.  The first
            # collective fires while the second half is still loading.
            bcasts = []
            for name, parts, k in (("a", parts_a, ka), ("b", parts_b, kb)):
                ps = psum.tile([1, 2 * k], FP32, tag=f"ps_{name}")
                nc.tensor.matmul(ps, ones_t, parts, start=True, stop=True)
                cc_sb = small.tile([1, 8], FP32, tag=f"cc_sb_{name}")
                nc.vector.memset(cc_sb, 0.0)
                nc.vector.reduce_sum(
                    out=cc_sb[:, 0:2],
                    in_=ps.rearrange("p (g k) -> p g k", g=2),
                    axis=mybir.AxisListType.X,
                )
                cc_in = dram.tile([1, 8], FP32, tag=f"cc_in_{name}")
                cc_out = dram.tile([1, 8], FP32, tag=f"cc_out_{name}")
                # gpsimd (SWDGE) queue: the sync HWDGE FIFO is busy with the
                # 16 MiB of loads and would delay this 32 B transfer (and
                # with it the collective trigger) until all loads drain.
                nc.gpsimd.dma_start(out=cc_in[:], in_=cc_sb)
                nc.gpsimd.collective_compute(
                    "AllReduce",
                    mybir.AluOpType.add,
                    replica_groups=replica,
                    ins=[cc_in.opt()],
                    outs=[cc_out.opt()],
                )
                # broadcast [S, SS] of this half to all 128 partitions
                cc_ap = cc_out[:]
                bc_src = bass.AP(
                    tensor=cc_ap.tensor, offset=cc_ap.offset, ap=[[0, P], [1, 2]]
                )
                bc = small.tile([P, 2], FP32, tag=f"bcast_{name}")
                nc.gpsimd.dma_start(out=bc, in_=bc_src)
                bcasts.append(bc)

            stats = small.tile([P, 2], FP32)            # [S, SS] global
            nc.vector.tensor_add(out=stats, in0=bcasts[0], in1=bcasts[1])
            S = stats[:, 0:1]
            SS = stats[:, 1:2]

            mean = small.tile([P, 1], FP32)             # ACT: S/cnt
            nc.scalar.activation(
                out=mean, in_=S, func=mybir.ActivationFunctionType.Copy,
                scale=1.0 / CNT,
            )
            t0 = small.tile([P, 1], FP32)               # DVE: S*S
            nc.vector.tensor_mul(out=t0, in0=S, in1=S)
            e2 = small.tile([P, 1], FP32)               # DVE: SS - S²/cnt
            nc.vector.tensor_scalar(
                out=e2, in0=t0, scalar1=-1.0 / CNT, scalar2=SS,
                op0=mybir.AluOpType.mult, op1=mybir.AluOpType.add,
            )
            std = small.tile([P, 1], FP32)              # ACT: sqrt(E/(cnt-1)+eps)
            nc.scalar.activation(
                out=std, in_=e2, func=mybir.ActivationFunctionType.Sqrt,
                scale=1.0 / (CNT - 1), bias=eps_t,
            )
            rstd = small.tile([P, 1], FP32)             # DVE
            nc.vector.reciprocal(out=rstd, in_=std)
            A_t = small.tile([P, 2], FP32)              # DVE: w * rstd
            nc.vector.tensor_scalar_mul(out=A_t, in0=w_t, scalar1=rstd)
            mA = small.tile([P, 2], FP32)               # ACT: A * mean
            nc.scalar.activation(
                out=mA, in_=A_t, func=mybir.ActivationFunctionType.Copy, scale=mean,
            )
            B_t = small.tile([P, 2], FP32)              # DVE: b - mean * A
            nc.vector.tensor_sub(out=B_t, in0=b_t, in1=mA)

            # half-size chunks first: the first store DMA launches ~1µs sooner
            norm_order = sorted(range(len(CHUNKS)), key=lambda ci: CHUNKS[ci][2])
            for ci in norm_order:
                t, c0, clen = CHUNKS[ci]
                col = t % 2
                xt = chunk_tiles[ci]
                nc.vector.tensor_scalar(
                    out=xt,
                    in0=xt,
                    scalar1=A_t[:, col : col + 1],
                    scalar2=B_t[:, col : col + 1],
                    op0=mybir.AluOpType.mult,
                    op1=mybir.AluOpType.add,
                )
                nc.sync.dma_start(out=ov[t, :, c0 : c0 + clen], in_=xt)

    nc.compile()
    return nc


_NC_CACHE: list = []


def _get_nc() -> bass.Bass:
    if not _NC_CACHE:
        _NC_CACHE.append(build_nc())
    return _NC_CACHE[0]


def kernel(x, weight, bias, indexes=None, **_unused):
    x = np.ascontiguousarray(np.asarray(x, dtype=np.float32))
    weight = np.ascontiguousarray(np.asarray(weight, dtype=np.float32).reshape(1, C, 1, 1))
    bias = np.ascontiguousarray(np.asarray(bias, dtype=np.float32).reshape(1, C, 1, 1))
    assert x.shape == (N, C, H, W)

    nc = _get_nc()
    in_maps = [
        {
            "x": np.ascontiguousarray(x[i * N_LOC : (i + 1) * N_LOC]),
            "weight": weight,
            "bias": bias,
        }
        for i in range(N_CORES)
    ]
    res = run_bass_kernel_spmd(nc, in_maps, core_ids=list(range(N_CORES)))
    out = np.concatenate([res.results[i]["out"] for i in range(N_CORES)], axis=0)
    return out


if __name__ == "__main__":
    nc = build_nc()
    print("build + compile OK:", nc)
